# revision 1
# baseline (speedup 1.0000x reference)
"""GraphVAE MPM kernel for Trainium2 (Bass/Tile), self-contained.

Math: the reference's S[i,j,a,b] tensor is separable off the overrides:
S = c_ij * Q[a,b] with c in {0,1}, so the per-iteration O(N^4) masked
max-product collapses to an O(N^3) grouped max (T1[j,a] = max_b Qz[a,b]*X[j,b],
clamped by G[j] = -1e6*min_{b>=R} X[j,b]) plus a 64x64 matmul with Cz.
Edge terms outside the real-node block reduce to per-row scalars built from
G via masked partition-sums (done as PE matmuls against static 0/1 matrices).

All 20 iterations run fully unrolled out of SBUF on each core (inputs are
tiny); the same program is replicated SPMD on all 8 cores and core 0's
output is returned.
"""

import numpy as np

N = 64
R = 56
ITERS = 20
BIGNEG = -3.0e38
BIGPOS = 3.0e38

_CACHE = {}


def _precompute(A_gt, vec_logits):
    """Host-side O(N^2) constant construction (mirrors reference's setup)."""
    A_gt = np.asarray(A_gt, np.float32)
    vec = np.asarray(vec_logits, np.float32)
    d = np.arange(N)

    iu = np.triu_indices(N, k=1)
    logits = np.zeros((N, N), np.float32)
    logits[iu] = vec
    logits = logits + logits.T
    logits[d, d] = np.float32(-10.0)
    B = (1.0 / (1.0 + np.exp(-logits))).astype(np.float32)

    A = A_gt.copy()
    r = int((A.sum(1) > 0).sum())
    real = d < r
    A[d, d] = np.where(real, np.float32(1.0), A[d, d])
    Bm = B.copy()
    Bm[d, d] = np.where(real, np.float32(1.0), Bm[d, d])
    dA = np.diagonal(A).copy()
    dB = np.diagonal(Bm).copy()
    degA = A.sum(1)
    degB = Bm.sum(1)
    node_sim = (1.0 / (np.abs(degA[:, None] - degB[None, :]) + 1.0)).astype(np.float32)

    Qz = (Bm * dB[:, None] * dB[None, :]).astype(np.float32)
    np.fill_diagonal(Qz, 0.0)
    qzv = np.ascontiguousarray(Qz[:R, :R]).reshape(-1)  # [R*R], a-major

    Cz = (A * dA[:, None] * dA[None, :]).astype(np.float32)
    np.fill_diagonal(Cz, 0.0)
    Cz[:, R:] = 0.0
    Cz[R:, :] = 0.0

    ns = (dA[:, None] * dB[None, :] * node_sim).astype(np.float32)
    mask2 = (d[:, None] < R) & (d[None, :] < R)
    nsm = np.where(mask2, ns, np.float32(-1e6)).astype(np.float32)

    # per-partition consts: col0 = kstat (0 if j<R else BIGNEG) for H=max(G,kstat)
    #                       col1 = kstat2 (0 if i<R else BIGPOS) for tsel=min(Gn,kstat2)
    pv = np.zeros((N, 2), np.float32)
    pv[R:, 0] = BIGNEG
    pv[R:, 1] = BIGPOS

    return {
        "qzv": qzv.astype(np.float32),
        "czp": Cz.astype(np.float32),
        "nsmp": nsm.astype(np.float32),
        "pvp": pv.astype(np.float32),
    }


def _build(iters=ITERS):
    import concourse.bass as bass
    import concourse.mybir as mybir
    from concourse import bacc
    from concourse.tile import TileContext

    f32 = mybir.dt.float32
    ALU = mybir.AluOpType
    ACTF = mybir.ActivationFunctionType
    AX = mybir.AxisListType

    nc = bacc.Bacc()
    qzv = nc.declare_dram_parameter("qzv", [R * R], f32, isOutput=False)
    czp = nc.declare_dram_parameter("czp", [N, N], f32, isOutput=False)
    nsmp = nc.declare_dram_parameter("nsmp", [N, N], f32, isOutput=False)
    pvp = nc.declare_dram_parameter("pvp", [N, 2], f32, isOutput=False)
    xoutp = nc.declare_dram_parameter("xout", [N, N], f32, isOutput=True)

    with TileContext(nc) as tc:
        with (
            tc.tile_pool(name="consts", bufs=1) as cp,
            tc.tile_pool(name="big", bufs=2) as bp,
            tc.tile_pool(name="sm", bufs=2) as sp,
            tc.tile_pool(name="ps", bufs=2, space="PSUM") as pp,
        ):
            # ---- constants ----
            qz = cp.tile([N, R * R], f32, name="qz")
            src = qzv[:]
            nc.sync.dma_start(
                out=qz,
                in_=bass.AP(tensor=src.tensor, offset=src.offset,
                            ap=[[0, N], list(src.ap[0])]),
            )
            cz = cp.tile([N, N], f32, name="cz")
            nc.sync.dma_start(out=cz, in_=czp[:])
            nsm = cp.tile([N, N], f32, name="nsm")
            nc.sync.dma_start(out=nsm, in_=nsmp[:])
            pv = cp.tile([N, 2], f32, name="pv")
            nc.sync.dma_start(out=pv, in_=pvp[:])

            ones = cp.tile([N, N], f32, name="ones")
            nc.vector.memset(ones, 1.0)
            selin = cp.tile([N, N], f32, name="selin")
            nc.vector.memset(selin, 0.0)
            nc.vector.memset(selin[:, 0:R], 1.0)
            selout = cp.tile([N, N], f32, name="selout")
            nc.vector.memset(selout, 0.0)
            nc.vector.memset(selout[:, R:N], 1.0)

            x = cp.tile([N, N], f32, name="x0")
            nc.vector.memset(x, 1.0 / N)
            tmin = cp.tile([N, 1], f32, name="tmin0")
            nc.vector.memset(tmin, 1.0 / N)

            qz3 = bass.AP(tensor=qz.tensor, offset=qz.offset,
                          ap=[list(qz.ap[0]), [R, R], [1, R]])

            for it in range(iters):
                last = it == iters - 1
                # --- scalar chain (ACT/GPSIMD/PE), overlaps the big DVE ops ---
                g = sp.tile([N, 1], f32, tag="g", name=f"g{it}")
                nc.scalar.activation(g, tmin, ACTF.Copy, bias=0.0, scale=-1.0e6)
                gn = sp.tile([N, 1], f32, tag="gn", name=f"gn{it}")
                nc.scalar.activation(gn, tmin, ACTF.Copy, bias=0.0, scale=1.0e6)
                t0 = sp.tile([N, 1], f32, tag="t0", name=f"t0{it}")
                nc.scalar.activation(t0, g, ACTF.Relu, bias=0.0, scale=1.0)
                tsel = sp.tile([N, 1], f32, tag="tsel", name=f"tsel{it}")
                nc.vector.tensor_tensor(tsel, gn, pv[:, 1:2], ALU.min)

                eb = pp.tile([N, 1], f32, tag="eb", name=f"eb{it}")
                nc.tensor.matmul(eb, selin, t0, start=True, stop=False)
                nc.tensor.matmul(eb, selout, g, start=False, stop=True)
                gt = pp.tile([N, 1], f32, tag="gt", name=f"gt{it}")
                nc.tensor.matmul(gt, ones, g, start=True, stop=True)

                e_sel = sp.tile([N, 1], f32, tag="e_sel", name=f"esel{it}")
                nc.scalar.activation(e_sel, eb, ACTF.Identity, bias=tsel, scale=1.0)
                e_tail = sp.tile([N, 1], f32, tag="e_tail", name=f"etail{it}")
                nc.scalar.activation(e_tail, gt, ACTF.Identity, bias=gn, scale=1.0)

                # --- big ops (DVE) ---
                u = bp.tile([N, R, R], f32, tag="u", name=f"u{it}")
                xb = bass.AP(tensor=x.tensor, offset=x.offset,
                             ap=[list(x.ap[0]), [0, R], [1, R]])
                nc.vector.tensor_tensor(u, xb, qz3, ALU.mult)
                t1 = sp.tile([N, R], f32, tag="t1", name=f"t1{it}")
                nc.vector.tensor_reduce(t1, u, AX.X, ALU.max)
                dt = sp.tile([N, R], f32, tag="dt", name=f"dt{it}")
                nc.vector.tensor_scalar(dt, t1, g, t0, ALU.max, ALU.subtract)

                m = pp.tile([N, R], f32, tag="m", name=f"m{it}")
                nc.tensor.matmul(m, cz, dt, start=True, stop=True)
                p1 = sp.tile([N, R], f32, tag="p1", name=f"p1{it}")
                nc.scalar.activation(p1, m, ACTF.Identity, bias=e_sel, scale=1.0)

                # --- assemble Xn ---
                xna = sp.tile([N, R], f32, tag="xna", name=f"xna{it}")
                nc.vector.tensor_tensor(xna, x[:, 0:R], nsm[:, 0:R], ALU.mult)
                xn = sp.tile([N, N], f32, tag="xn", name=f"xn{it}")
                nc.vector.tensor_tensor(xn[:, 0:R], xna, p1, ALU.add)
                nc.vector.tensor_scalar(xn[:, R:N], x[:, R:N], -1.0e6, e_tail,
                                        ALU.mult, ALU.add)

                # --- normalize ---
                scr = sp.tile([N, N], f32, tag="scr", name=f"scr{it}")
                qrow = sp.tile([N, 1], f32, tag="qrow", name=f"qrow{it}")
                nc.scalar.activation(scr, xn, ACTF.Square, bias=0.0, scale=1.0,
                                     accum_out=qrow)
                npsum = pp.tile([N, 1], f32, tag="npsum", name=f"np{it}")
                nc.tensor.matmul(npsum, ones, qrow, start=True, stop=True)
                sn = sp.tile([N, 1], f32, tag="sn", name=f"sn{it}")
                nc.scalar.activation(sn, npsum, ACTF.Sqrt, bias=0.0, scale=1.0)
                invn = sp.tile([N, 1], f32, tag="invn", name=f"invn{it}")
                nc.vector.reciprocal(invn, sn)

                xnew = sp.tile([N, N], f32, tag="xnew", name=f"xnew{it}")
                nc.vector.tensor_scalar(xnew[:, 0:R], xn[:, 0:R], invn, None,
                                        ALU.mult)
                if last:
                    nc.vector.tensor_scalar(xnew[:, R:N], xn[:, R:N], invn,
                                            None, ALU.mult)
                else:
                    tmin2 = sp.tile([N, 1], f32, tag="tmin", name=f"tmin{it}")
                    nc.vector.tensor_scalar(xnew[:, R:N], xn[:, R:N], invn,
                                            None, ALU.mult, ALU.min,
                                            accum_out=tmin2)
                    tmin = tmin2
                x = xnew

            nc.sync.dma_start(out=xoutp[:], in_=x)

    nc.finalize()
    return nc


def _get_nc(iters=ITERS):
    key = ("nc", iters)
    if key not in _CACHE:
        _CACHE[key] = _build(iters)
    return _CACHE[key]


def kernel(A_gt, vec_logits, R_int):
    assert int(R_int) == R and A_gt.shape == (N, N)
    ins = _precompute(A_gt, vec_logits)
    nc = _get_nc()

    from concourse.bass_utils import run_bass_kernel_spmd

    core_ids = list(range(8))
    res = run_bass_kernel_spmd(nc, [dict(ins) for _ in core_ids], core_ids)
    out = np.asarray(res.results[0]["xout"], dtype=np.float32).reshape(N, N)
    return out



# revision 2
# speedup vs baseline: 112.5777x; 112.5777x over previous
"""GraphVAE MPM kernel for Trainium2 (Bass/Tile), self-contained.

Math: the reference's S[i,j,a,b] tensor is separable off the overrides:
S = c_ij * Q[a,b] with c in {0,1}, so the per-iteration O(N^4) masked
max-product collapses to an O(N^3) grouped max (T1[j,a] = max_b Qz[a,b]*X[j,b],
clamped by G[j] = -1e6*min_{b>=R} X[j,b]) plus a 64x64 matmul with Cz.
Edge terms outside the real-node block reduce to per-row scalars built from
G via masked partition-sums (done as PE matmuls against static 0/1 matrices).

All 20 iterations run fully unrolled out of SBUF on each core (inputs are
tiny); the same program is replicated SPMD on all 8 cores and core 0's
output is returned.
"""

import numpy as np

N = 64
R = 56
ITERS = 20
BIGNEG = -3.0e38
BIGPOS = 3.0e38

_CACHE = {}


def _precompute(A_gt, vec_logits):
    """Host-side O(N^2) constant construction (mirrors reference's setup)."""
    A_gt = np.asarray(A_gt, np.float32)
    vec = np.asarray(vec_logits, np.float32)
    d = np.arange(N)

    iu = np.triu_indices(N, k=1)
    logits = np.zeros((N, N), np.float32)
    logits[iu] = vec
    logits = logits + logits.T
    logits[d, d] = np.float32(-10.0)
    B = (1.0 / (1.0 + np.exp(-logits))).astype(np.float32)

    A = A_gt.copy()
    r = int((A.sum(1) > 0).sum())
    real = d < r
    A[d, d] = np.where(real, np.float32(1.0), A[d, d])
    Bm = B.copy()
    Bm[d, d] = np.where(real, np.float32(1.0), Bm[d, d])
    dA = np.diagonal(A).copy()
    dB = np.diagonal(Bm).copy()
    degA = A.sum(1)
    degB = Bm.sum(1)
    node_sim = (1.0 / (np.abs(degA[:, None] - degB[None, :]) + 1.0)).astype(np.float32)

    Qz = (Bm * dB[:, None] * dB[None, :]).astype(np.float32)
    np.fill_diagonal(Qz, 0.0)
    qzv = np.ascontiguousarray(Qz[:R, :R]).reshape(-1)  # [R*R], a-major

    Cz = (A * dA[:, None] * dA[None, :]).astype(np.float32)
    np.fill_diagonal(Cz, 0.0)
    Cz[:, R:] = 0.0
    Cz[R:, :] = 0.0

    ns = (dA[:, None] * dB[None, :] * node_sim).astype(np.float32)
    mask2 = (d[:, None] < R) & (d[None, :] < R)
    nsm = np.where(mask2, ns, np.float32(-1e6)).astype(np.float32)

    # per-partition consts: col0 = kstat (0 if j<R else BIGNEG) for H=max(G,kstat)
    #                       col1 = kstat2 (0 if i<R else BIGPOS) for tsel=min(Gn,kstat2)
    pv = np.zeros((N, 2), np.float32)
    pv[R:, 0] = BIGNEG
    pv[R:, 1] = BIGPOS

    return {
        "qzv": qzv.astype(np.float32),
        "czp": Cz.astype(np.float32),
        "nsmp": nsm.astype(np.float32),
        "pvp": pv.astype(np.float32),
    }


def _build(iters=ITERS):
    import concourse.bass as bass
    import concourse.mybir as mybir
    from concourse import bacc
    from concourse.tile import TileContext

    f32 = mybir.dt.float32
    ALU = mybir.AluOpType
    ACTF = mybir.ActivationFunctionType
    AX = mybir.AxisListType

    nc = bacc.Bacc()
    qzv = nc.declare_dram_parameter("qzv", [R * R], f32, isOutput=False)
    czp = nc.declare_dram_parameter("czp", [N, N], f32, isOutput=False)
    nsmp = nc.declare_dram_parameter("nsmp", [N, N], f32, isOutput=False)
    pvp = nc.declare_dram_parameter("pvp", [N, 2], f32, isOutput=False)
    xoutp = nc.declare_dram_parameter("xout", [N, N], f32, isOutput=True)

    with TileContext(nc) as tc:
        with (
            tc.tile_pool(name="consts", bufs=1) as cp,
            tc.tile_pool(name="big", bufs=2) as bp,
            tc.tile_pool(name="sm", bufs=2) as sp,
            tc.tile_pool(name="ps", bufs=2, space="PSUM") as pp,
        ):
            # ---- constants ----
            qz = cp.tile([N, R * R], f32, name="qz")
            src = qzv[:]
            nc.sync.dma_start(
                out=qz,
                in_=bass.AP(tensor=src.tensor, offset=src.offset,
                            ap=[[0, N], list(src.ap[0])]),
            )
            cz = cp.tile([N, N], f32, name="cz")
            nc.sync.dma_start(out=cz, in_=czp[:])
            nsm = cp.tile([N, N], f32, name="nsm")
            nc.sync.dma_start(out=nsm, in_=nsmp[:])
            pv = cp.tile([N, 2], f32, name="pv")
            nc.sync.dma_start(out=pv, in_=pvp[:])

            ones = cp.tile([N, N], f32, name="ones")
            nc.vector.memset(ones, 1.0)
            selin = cp.tile([N, N], f32, name="selin")
            nc.vector.memset(selin, 0.0)
            nc.vector.memset(selin[:, 0:R], 1.0)
            selout = cp.tile([N, N], f32, name="selout")
            nc.vector.memset(selout, 0.0)
            nc.vector.memset(selout[:, R:N], 1.0)

            x = cp.tile([N, N], f32, name="x0")
            nc.vector.memset(x, 1.0 / N)
            tmin = cp.tile([N, 1], f32, name="tmin0")
            nc.vector.memset(tmin, 1.0 / N)

            qz3 = bass.AP(tensor=qz.tensor, offset=qz.offset,
                          ap=[list(qz.ap[0]), [R, R], [1, R]])

            with tc.For_i(0, iters, 1):
                # --- scalar chain (ACT/GPSIMD/PE), overlaps the big DVE ops ---
                g = sp.tile([N, 1], f32, tag="g", name="g")
                nc.scalar.activation(g, tmin, ACTF.Copy, bias=0.0, scale=-1.0e6)
                gn = sp.tile([N, 1], f32, tag="gn", name="gn")
                nc.scalar.activation(gn, tmin, ACTF.Copy, bias=0.0, scale=1.0e6)
                t0 = sp.tile([N, 1], f32, tag="t0", name="t0")
                nc.scalar.activation(t0, g, ACTF.Relu, bias=0.0, scale=1.0)
                tsel = sp.tile([N, 1], f32, tag="tsel", name="tsel")
                nc.vector.tensor_tensor(tsel, gn, pv[:, 1:2], ALU.min)

                eb = pp.tile([N, 1], f32, tag="eb", name="eb")
                nc.tensor.matmul(eb, selin, t0, start=True, stop=False)
                nc.tensor.matmul(eb, selout, g, start=False, stop=True)
                gt = pp.tile([N, 1], f32, tag="gt", name="gt")
                nc.tensor.matmul(gt, ones, g, start=True, stop=True)

                e_sel = sp.tile([N, 1], f32, tag="e_sel", name="esel")
                nc.scalar.activation(e_sel, eb, ACTF.Identity, bias=tsel, scale=1.0)
                e_tail = sp.tile([N, 1], f32, tag="e_tail", name="etail")
                nc.scalar.activation(e_tail, gt, ACTF.Identity, bias=gn, scale=1.0)

                # --- big ops (DVE) ---
                u = bp.tile([N, R, R], f32, tag="u", name="u")
                xb = bass.AP(tensor=x.tensor, offset=x.offset,
                             ap=[list(x.ap[0]), [0, R], [1, R]])
                nc.vector.tensor_tensor(u, xb, qz3, ALU.mult)
                t1 = sp.tile([N, R], f32, tag="t1", name="t1")
                nc.vector.tensor_reduce(t1, u, AX.X, ALU.max)
                dt = sp.tile([N, R], f32, tag="dt", name="dt")
                nc.vector.tensor_scalar(dt, t1, g, t0, ALU.max, ALU.subtract)

                m = pp.tile([N, R], f32, tag="m", name="m")
                nc.tensor.matmul(m, cz, dt, start=True, stop=True)
                p1 = sp.tile([N, R], f32, tag="p1", name="p1")
                nc.scalar.activation(p1, m, ACTF.Identity, bias=e_sel, scale=1.0)

                # --- assemble Xn ---
                xna = sp.tile([N, R], f32, tag="xna", name="xna")
                nc.vector.tensor_tensor(xna, x[:, 0:R], nsm[:, 0:R], ALU.mult)
                xn = sp.tile([N, N], f32, tag="xn", name="xn")
                nc.vector.tensor_tensor(xn[:, 0:R], xna, p1, ALU.add)
                nc.vector.tensor_scalar(xn[:, R:N], x[:, R:N], -1.0e6, e_tail,
                                        ALU.mult, ALU.add)

                # --- normalize ---
                scr = sp.tile([N, N], f32, tag="scr", name="scr")
                qrow = sp.tile([N, 1], f32, tag="qrow", name="qrow")
                nc.scalar.activation(scr, xn, ACTF.Square, bias=0.0, scale=1.0,
                                     accum_out=qrow)
                npsum = pp.tile([N, 1], f32, tag="npsum", name="np")
                nc.tensor.matmul(npsum, ones, qrow, start=True, stop=True)
                sn = sp.tile([N, 1], f32, tag="sn", name="sn")
                nc.scalar.activation(sn, npsum, ACTF.Sqrt, bias=0.0, scale=1.0)
                invn = sp.tile([N, 1], f32, tag="invn", name="invn")
                nc.vector.reciprocal(invn, sn)

                # write the normalized result back into the loop-carried x;
                # WAR on x (u/xna/xn-tail reads) is ordered by Tile tracking
                nc.vector.tensor_scalar(x[:, 0:R], xn[:, 0:R], invn, None,
                                        ALU.mult)
                nc.vector.tensor_scalar(x[:, R:N], xn[:, R:N], invn,
                                        None, ALU.mult, ALU.min,
                                        accum_out=tmin)

            nc.sync.dma_start(out=xoutp[:], in_=x)

    nc.finalize()
    return nc


def _get_nc(iters=ITERS):
    key = ("nc", iters)
    if key not in _CACHE:
        _CACHE[key] = _build(iters)
    return _CACHE[key]


def kernel(A_gt, vec_logits, R_int):
    assert int(R_int) == R and A_gt.shape == (N, N)
    ins = _precompute(A_gt, vec_logits)
    nc = _get_nc()

    from concourse.bass_utils import run_bass_kernel_spmd

    core_ids = list(range(8))
    res = run_bass_kernel_spmd(nc, [dict(ins) for _ in core_ids], core_ids)
    out = np.asarray(res.results[0]["xout"], dtype=np.float32).reshape(N, N)
    return out



# revision 8
# speedup vs baseline: 336.9919x; 2.9934x over previous
"""GraphVAE MPM kernel for Trainium2 (Bass/Tile), self-contained.

Math: the reference's S[i,j,a,b] tensor is separable off the overrides:
S = c_ij * Q[a,b] with c in {0,1}, so the per-iteration O(N^4) masked
max-product collapses to an O(N^3) grouped max (T1[j,a] = max_b Qz[a,b]*X[j,b],
clamped by G[j] = -1e6*min_{b>=R} X[j,b]) plus a 64x64 matmul with Cz.
Edge terms outside the real-node block reduce to per-row scalars built from
G via masked partition-sums (done as PE matmuls against static 0/1 matrices).

Key structural optimizations:
- The whole iteration map is positively 1-homogeneous in X, so the L2
  normalization can be applied with one iteration of LAG (any positive
  per-iteration scale preserves the final direction; one exact normalize
  after the loop).  This takes the norm chain (square/sum/rsqrt) off the
  per-iteration critical path.  The lagged scale itself is a bit-trick
  rsqrt (0x5f3759df), exact enough for range control; fp32 magnitudes
  stay within [1e-16, 2e31] (period-6 log oscillation, verified).
- The G clamp is fused into the big max-reduce by writing G into an extra
  trailing column of the product tensor (u is [N, R, R+1]).
- All small ops are placed on ACT/PE/GPSIMD so the DVE only runs the two
  big O(N^3) ops plus the x-update; no ACT function outside the
  exp_and_others table set is used inside the loop (Sqrt would force a
  ~2.7us table reload per iteration).
- The 20 iterations run as a For_i hardware loop (program size constant
  in the trip count) with UNROLL bodies per back-edge to amortize the
  ~2.7us all-engine barrier.

The same program is replicated SPMD on all 8 cores and core 0's output is
returned.
"""

import numpy as np

N = 64
R = 56
ITERS = 20
UNROLL = 4
BIGNEG = -3.0e38
RSQRT_MAGIC = 0x5F3759DF

_CACHE = {}


def _precompute(A_gt, vec_logits):
    """Host-side O(N^2) constant construction (mirrors reference's setup)."""
    A_gt = np.asarray(A_gt, np.float32)
    vec = np.asarray(vec_logits, np.float32)
    d = np.arange(N)

    iu = np.triu_indices(N, k=1)
    logits = np.zeros((N, N), np.float32)
    logits[iu] = vec
    logits = logits + logits.T
    logits[d, d] = np.float32(-10.0)
    B = (1.0 / (1.0 + np.exp(-logits))).astype(np.float32)

    A = A_gt.copy()
    r = int((A.sum(1) > 0).sum())
    real = d < r
    A[d, d] = np.where(real, np.float32(1.0), A[d, d])
    Bm = B.copy()
    Bm[d, d] = np.where(real, np.float32(1.0), Bm[d, d])
    dA = np.diagonal(A).copy()
    dB = np.diagonal(Bm).copy()
    degA = A.sum(1)
    degB = Bm.sum(1)
    node_sim = (1.0 / (np.abs(degA[:, None] - degB[None, :]) + 1.0)).astype(np.float32)

    Qz = (Bm * dB[:, None] * dB[None, :]).astype(np.float32)
    np.fill_diagonal(Qz, 0.0)
    qzv = np.ascontiguousarray(Qz[:R, :R]).reshape(-1)  # [R*R], a-major

    Cz = (A * dA[:, None] * dA[None, :]).astype(np.float32)
    np.fill_diagonal(Cz, 0.0)
    Cz[:, R:] = 0.0
    Cz[R:, :] = 0.0

    ns = (dA[:, None] * dB[None, :] * node_sim).astype(np.float32)
    mask2 = (d[:, None] < R) & (d[None, :] < R)
    nsm = np.where(mask2, ns, np.float32(-1e6)).astype(np.float32)

    # per-partition const: col0 = 0 if j<R else BIGNEG, for tsel = -(max(g, pv0))
    pv = np.zeros((N, 2), np.float32)
    pv[R:, 0] = BIGNEG

    return {
        "qzv": qzv.astype(np.float32),
        "czp": Cz.astype(np.float32),
        "nsmp": nsm.astype(np.float32),
        "pvp": pv.astype(np.float32),
    }


def _build(iters=ITERS):
    import concourse.bass as bass
    import concourse.mybir as mybir
    from concourse import bacc
    from concourse.tile import TileContext

    assert iters % UNROLL == 0
    f32 = mybir.dt.float32
    i32 = mybir.dt.int32
    ALU = mybir.AluOpType
    ACTF = mybir.ActivationFunctionType
    AX = mybir.AxisListType

    nc = bacc.Bacc()
    qzv = nc.declare_dram_parameter("qzv", [R * R], f32, isOutput=False)
    czp = nc.declare_dram_parameter("czp", [N, N], f32, isOutput=False)
    nsmp = nc.declare_dram_parameter("nsmp", [N, N], f32, isOutput=False)
    pvp = nc.declare_dram_parameter("pvp", [N, 2], f32, isOutput=False)
    xoutp = nc.declare_dram_parameter("xout", [N, N], f32, isOutput=True)

    with TileContext(nc) as tc:
        with (
            tc.tile_pool(name="consts", bufs=1) as cp,
            tc.tile_pool(name="big", bufs=2) as bp,
            tc.tile_pool(name="sm", bufs=2) as sp,
            tc.tile_pool(name="ps", bufs=2, space="PSUM") as pp,
        ):
            # ---- constants ----
            qz = cp.tile([N, R * R], f32, name="qz")
            src = qzv[:]
            nc.sync.dma_start(
                out=qz,
                in_=bass.AP(tensor=src.tensor, offset=src.offset,
                            ap=[[0, N], list(src.ap[0])]),
            )
            cz = cp.tile([N, N], f32, name="cz")
            nc.sync.dma_start(out=cz, in_=czp[:])
            nsm = cp.tile([N, N], f32, name="nsm")
            nc.sync.dma_start(out=nsm, in_=nsmp[:])
            pv = cp.tile([N, 2], f32, name="pv")
            nc.sync.dma_start(out=pv, in_=pvp[:])

            ones = cp.tile([N, N], f32, name="ones")
            nc.vector.memset(ones, 1.0)
            selin = cp.tile([N, N], f32, name="selin")
            nc.vector.memset(selin, 0.0)
            nc.vector.memset(selin[:, 0:R], 1.0)
            selout = cp.tile([N, N], f32, name="selout")
            nc.vector.memset(selout, 0.0)
            nc.vector.memset(selout[:, R:N], 1.0)

            # loop-carried state
            x = cp.tile([N, N], f32, name="x0")
            nc.vector.memset(x, 1.0 / N)
            tmin = cp.tile([N, 1], f32, name="tmin0")
            nc.vector.memset(tmin, 1.0 / N)
            invn = cp.tile([N, 1], f32, name="invn0")
            nc.vector.memset(invn, 1.0)

            qz3 = bass.AP(tensor=qz.tensor, offset=qz.offset,
                          ap=[list(qz.ap[0]), [R, R], [1, R]])

            def body():
                # --- per-row scalar chain (ACT/GPSIMD/PE) ---
                g = sp.tile([N, 1], f32, tag="g", name="g")
                nc.scalar.activation(g, tmin, ACTF.Copy, bias=0.0, scale=-1.0e6)
                gn = sp.tile([N, 1], f32, tag="gn", name="gn")
                nc.scalar.activation(gn, tmin, ACTF.Copy, bias=0.0, scale=1.0e6)
                t0 = sp.tile([N, 1], f32, tag="t0", name="t0")
                nc.scalar.activation(t0, g, ACTF.Relu, bias=0.0, scale=1.0)
                # tsel = -(max(g, pv0)): = -t0 for i<R, = -g (=gn) for i>=R
                tsel = sp.tile([N, 1], f32, tag="tsel", name="tsel")
                nc.gpsimd.tensor_scalar(tsel, g, pv[:, 0:1], -1.0,
                                        ALU.max, ALU.mult)

                sc = pp.tile([N, 3], f32, tag="sc", name="sc")
                nc.tensor.matmul(sc[:, 0:1], selin, t0, start=True, stop=False)
                nc.tensor.matmul(sc[:, 0:1], selout, g, start=False, stop=True)
                nc.tensor.matmul(sc[:, 1:2], ones, g, start=True, stop=True)
                nc.tensor.matmul(sc[:, 2:3], cz, t0, start=True, stop=True)

                e_sel = sp.tile([N, 1], f32, tag="e_sel", name="esel")
                nc.scalar.activation(e_sel, sc[:, 0:1], ACTF.Identity,
                                     bias=tsel, scale=1.0)
                esel2 = sp.tile([N, 1], f32, tag="esel2", name="esel2")
                nc.scalar.activation(esel2, sc[:, 2:3], ACTF.Identity,
                                     bias=e_sel, scale=-1.0)
                e_tail = sp.tile([N, 1], f32, tag="e_tail", name="etail")
                nc.scalar.activation(e_tail, sc[:, 1:2], ACTF.Identity,
                                     bias=gn, scale=1.0)

                # --- big ops (DVE): u = x*qz with G in a trailing column,
                # then one max-reduce gives the clamped T1 directly ---
                u = bp.tile([N, R * (R + 1)], f32, tag="u", name="u")
                u_gcol = bass.AP(tensor=u.tensor, offset=u.offset + R,
                                 ap=[list(u.ap[0]), [R + 1, R]])
                nc.scalar.activation(u_gcol, ones[:, 0:R], ACTF.Identity,
                                     bias=g, scale=0.0)
                u_main = bass.AP(tensor=u.tensor, offset=u.offset,
                                 ap=[list(u.ap[0]), [R + 1, R], [1, R]])
                xb = bass.AP(tensor=x.tensor, offset=x.offset,
                             ap=[list(x.ap[0]), [0, R], [1, R]])
                nc.vector.tensor_tensor(u_main, xb, qz3, ALU.mult)
                u_all = bass.AP(tensor=u.tensor, offset=u.offset,
                                ap=[list(u.ap[0]), [R + 1, R], [1, R + 1]])
                t1 = sp.tile([N, R], f32, tag="t1", name="t1")
                nc.vector.tensor_reduce(t1, u_all, AX.X, ALU.max)

                m = pp.tile([N, R], f32, tag="m", name="m")
                nc.tensor.matmul(m, cz, t1, start=True, stop=True)

                # --- assemble Xn ---
                xna = sp.tile([N, R], f32, tag="xna", name="xna")
                nc.gpsimd.tensor_tensor(xna, x[:, 0:R], nsm[:, 0:R], ALU.mult)
                xna2 = sp.tile([N, R], f32, tag="xna2", name="xna2")
                nc.scalar.activation(xna2, xna, ACTF.Identity,
                                     bias=esel2, scale=1.0)
                xn = sp.tile([N, N], f32, tag="xn", name="xn")
                nc.vector.tensor_tensor(xn[:, 0:R], m, xna2, ALU.add)
                nc.scalar.activation(xn[:, R:N], x[:, R:N], ACTF.Identity,
                                     bias=e_tail, scale=-1.0e6)

                # --- x update with LAGGED scale (invn from previous body) ---
                nc.vector.tensor_scalar(x[:, 0:R], xn[:, 0:R], invn, None,
                                        ALU.mult)
                nc.vector.tensor_scalar(x[:, R:N], xn[:, R:N], invn,
                                        None, ALU.mult, ALU.min,
                                        accum_out=tmin)

                # --- next body's scale: bit-trick rsqrt of sum(xn^2) ---
                scr = sp.tile([N, N], f32, tag="scr", name="scr")
                qrow = sp.tile([N, 1], f32, tag="qrow", name="qrow")
                nc.scalar.activation(scr, xn, ACTF.Square, bias=0.0,
                                     scale=1.0, accum_out=qrow)
                nps = pp.tile([N, 1], f32, tag="np", name="np")
                nc.tensor.matmul(nps, ones, qrow, start=True, stop=True)
                scop = sp.tile([N, 1], f32, tag="scop", name="scop")
                nc.scalar.activation(scop, nps, ACTF.Copy, bias=0.0, scale=1.0)
                nb = sp.tile([N, 1], i32, tag="nb", name="nb")
                nc.vector.tensor_scalar(nb, scop.bitcast(i32), 1, -1,
                                        ALU.logical_shift_right,
                                        ALU.bitwise_xor)
                nc.vector.tensor_single_scalar(invn.bitcast(i32), nb,
                                               RSQRT_MAGIC + 1, ALU.add)

            with tc.For_i(0, iters // UNROLL, 1):
                for _ in range(UNROLL):
                    body()

            # --- final exact normalization (homogeneity: one true L2 norm) ---
            scrf = sp.tile([N, N], f32, tag="scr", name="scrf")
            qrowf = sp.tile([N, 1], f32, tag="qrow", name="qrowf")
            nc.scalar.activation(scrf, x, ACTF.Square, bias=0.0, scale=1.0,
                                 accum_out=qrowf)
            npf = pp.tile([N, 1], f32, tag="np", name="npf")
            nc.tensor.matmul(npf, ones, qrowf, start=True, stop=True)
            snf = sp.tile([N, 1], f32, tag="scop", name="snf")
            nc.scalar.activation(snf, npf, ACTF.Sqrt, bias=0.0, scale=1.0)
            invf = sp.tile([N, 1], f32, tag="nb", name="invf")
            nc.vector.reciprocal(invf, snf)
            xo = sp.tile([N, N], f32, tag="xn", name="xo")
            nc.vector.tensor_scalar(xo, x, invf, None, ALU.mult)
            nc.sync.dma_start(out=xoutp[:], in_=xo)

    nc.finalize()
    return nc


def _get_nc(iters=ITERS):
    key = ("nc", iters)
    if key not in _CACHE:
        _CACHE[key] = _build(iters)
    return _CACHE[key]


def kernel(A_gt, vec_logits, R_int):
    assert int(R_int) == R and A_gt.shape == (N, N)
    ins = _precompute(A_gt, vec_logits)
    nc = _get_nc()

    from concourse.bass_utils import run_bass_kernel_spmd

    core_ids = list(range(8))
    res = run_bass_kernel_spmd(nc, [dict(ins) for _ in core_ids], core_ids)
    out = np.asarray(res.results[0]["xout"], dtype=np.float32).reshape(N, N)
    return out
